# revision 23
# baseline (speedup 1.0000x reference)
"""Trainium2 Bass kernel for nn_CANN_75857712382071.

Single-head self-attention (B=32, A=2048, D=128) with scalar output
projection, algebraically collapsed:

    out[b,aq] = (sum_ak E * (w+c+bo)) / (sum_ak E)
    E = exp(scale * (z M z^T + 1 (x) g)),  M = Wq^T Wk
    g[ak] = z[ak] . (Wk^T bq),   w[ak] = z[ak] . (Wv^T Wo^T)

q/k/v/h are never materialized; softmax max-subtraction is skipped
(logits are O(10); softmax is shift-invariant in exact arithmetic).

Data-parallel over batch: 4 batches per core on 8 NeuronCores.
Batches are software-pipelined: batch b+1's setup (z DMA, PE
transposes to zT, UT = M zT + gw, w column) is emitted in small pieces
inside batch b's main loop so ScalarE (exp) never starves.
"""

import sys
import types

import numpy as np

N_CORES = 8
B, A, D = 32, 2048, 128
B_PER = B // N_CORES
SCALE = float(D) ** -0.5
SCORES_DTYPE = "bf16"   # "bf16" (fast) or "f32r" (more precise scores)


def _install_axon_shim():
    """Allow run_bass_kernel_spmd(trace=True) to NTFF-profile under axon."""
    try:
        import antenv  # noqa: F401
    except ImportError:
        return
    if "antenv.axon_hooks" not in sys.modules:
        mod = types.ModuleType("antenv.axon_hooks")
        _hook = [None]
        mod.set_axon_ntff_profile_hook = lambda h: _hook.__setitem__(0, h)
        mod.get_axon_ntff_profile_hook = lambda: _hook[0]
        sys.modules["antenv.axon_hooks"] = mod
    from antenv.axon_hooks import (
        get_axon_ntff_profile_hook,
        set_axon_ntff_profile_hook,
    )
    if get_axon_ntff_profile_hook() is None:
        try:
            from trn_agent_boot.trn_boot import _ntff_profile_via_ctypes
            set_axon_ntff_profile_hook(
                _ntff_profile_via_ctypes("/opt/axon/libaxon_pjrt.so"))
        except Exception:
            pass
    try:
        from concourse import bass_utils
        bass_utils.upload_artifacts = lambda tmpdir: tmpdir
    except Exception:
        pass


def _build_program(cbo: float, scores_bf16: bool):
    import concourse.bacc as bacc
    import concourse.mybir as mybir
    import concourse.tile as tile
    from concourse import masks

    f32 = mybir.dt.float32
    f32r = mybir.dt.float32r
    bf16 = mybir.dt.bfloat16
    sdt = bf16 if scores_bf16 else f32r
    AF = mybir.ActivationFunctionType
    ADD = mybir.AluOpType.add
    MULT = mybir.AluOpType.mult

    nc = bacc.Bacc("TRN2", target_bir_lowering=False, debug=False,
                   num_devices=N_CORES, num_swdge_queues=2)

    z_d = nc.dram_tensor("z", [B_PER, A, D], f32, kind="ExternalInput").ap()
    m_d = nc.dram_tensor("m_lhs", [D, D], f32, kind="ExternalInput").ap()
    gw_d = nc.dram_tensor("gw", [D, 1], f32, kind="ExternalInput").ap()
    wv_d = nc.dram_tensor("wv", [D, 2], f32, kind="ExternalInput").ap()
    out_d = nc.dram_tensor("out", [B_PER, A], f32, kind="ExternalOutput").ap()

    NT = A // 128          # 16 ak tiles / z tiles
    NH = A // 1024         # 2 aq halves (ACT op width 1024)
    NC_ = A // 512         # 4 aq chunks (nd accumulators)

    with tile.TileContext(nc) as tc:
        with (
            tc.tile_pool(name="sb", bufs=1) as sb,
            tc.tile_pool(name="ps_sc", bufs=2, space="PSUM") as ps_sc,
            tc.tile_pool(name="ps_nd", bufs=1, space="PSUM") as ps_nd,
        ):
            sbc = sbz = sbe = sbb = sb
            # ---- constants ----
            m_f = sbc.tile([D, D], f32)
            nc.sync.dma_start(m_f[:], m_d[:])
            gw_col = sbc.tile([D, 1], f32)
            nc.sync.dma_start(gw_col[:], gw_d[:])
            wv_f = sbc.tile([D, 2], f32)
            nc.sync.dma_start(wv_f[:], wv_d[:])
            ident = sbc.tile([D, D], f32)
            masks.make_identity(nc, ident[:])
            m_r = sbc.tile([D, D], sdt)
            nc.vector.tensor_copy(m_r[:], m_f[:])
            wv_r = sbc.tile([D, 2], sdt)
            nc.vector.tensor_copy(wv_r[:], wv_f[:])

            # ACT table warmup (overlaps first z DMAs)
            warm = sbc.tile([D, 1], f32)
            nc.scalar.activation(warm[:], gw_col[:], AF.Exp, scale=0.0)

            # PE HAM pre-warm: dummy bf16 matmuls on junk data while the
            # first z DMA is in flight, so the prologue runs at 2.4 GHz
            junk = sbc.tile([128, 512], bf16)
            nc.gpsimd.memset(junk[:, 0:8], 0.0)
            pjunk = ps_sc.tile([128, 512], f32, name="pjunk", tag="sc")
            for i in range(14):
                nc.tensor.matmul(pjunk[:], junk[:, 0:128], junk[:],
                                 start=True, stop=True)

            st = {}  # per-batch live tiles

            def emit_z_dmas(b):
                s = st.setdefault(b, {})
                s["zT"] = sbb.tile([D, A], sdt, name=f"zT{b}", tag="zT", bufs=2)
                zn = sbz.tile([128, A], f32, name=f"zn{b}", tag="zn", bufs=2)
                zsrc = z_d[b].rearrange("(t p) d -> p t d", p=128)
                zdst = zn.rearrange("p (t d) -> p t d", d=D)
                engs = (nc.sync, nc.scalar, nc.gpsimd, nc.gpsimd)
                for q in range(8):
                    engs[q % 4].dma_start(zdst[:, 2 * q:2 * q + 2],
                                          zsrc[:, 2 * q:2 * q + 2])
                s["zn"] = zn

            def emit_cast_half(b, h):
                s = st[b]
                if "znb" not in s:
                    s["znb"] = sbz.tile([128, A], bf16, name=f"znb{b}",
                                        tag="znb", bufs=2)
                nc.vector.tensor_copy(s["znb"][:, h * 1024:(h + 1) * 1024],
                                      s["zn"][:, h * 1024:(h + 1) * 1024])

            def emit_transpose_dma(b, h):
                # 8 xbar DMA transposes straight into zT (no PE, no PSUM)
                s = st[b]
                for i in range(8 * h, 8 * h + 8):
                    eng = nc.sync if i % 2 == 0 else nc.scalar
                    eng.dma_start_transpose(
                        out=s["zT"][:, i * 128:(i + 1) * 128],
                        in_=s["znb"][:, i * 128:(i + 1) * 128])

            def emit_transpose_group(b, g, n=2):
                # n transposes into one psum slot, then 1 copy
                s = st[b]
                pt = ps_sc.tile([128, 1024], f32, name=f"pt{b}_{g}", tag="sc")
                for j in range(n):
                    i = n * g + j
                    nc.tensor.transpose(pt[:, j * 128:(j + 1) * 128],
                                        s["zn"][:, i * 128:(i + 1) * 128],
                                        ident[:])
                nc.vector.tensor_copy(
                    s["zT"][:, g * n * 128:(g + 1) * n * 128],
                    pt[:, 0:n * 128])

            def emit_ut_half(b, h):
                s = st[b]
                zT = s["zT"]
                if h == 0:
                    s["UT"] = sbb.tile([D, A], sdt, name=f"UT{b}", tag="UT",
                                       bufs=2)
                UT = s["UT"]
                pu = ps_sc.tile([128, 1024], f32, name=f"pu{b}_{h}",
                                tag="sc")
                for j in range(2):
                    o = h * 1024 + j * 512
                    nc.tensor.matmul(pu[:, j * 512:(j + 1) * 512],
                                     m_r[:], zT[:, o:o + 512],
                                     start=True, stop=True)
                nc.vector.tensor_scalar(
                    UT[:, h * 1024:(h + 1) * 1024], pu[:], gw_col[:],
                    None, ADD)

            def emit_w(b):
                s = st[b]
                zT = s["zT"]
                pw = ps_sc.tile([128, 2 * NT], f32, name=f"pw{b}", tag="sc")
                for t in range(NT):
                    nc.tensor.matmul(pw[:, 2 * t:2 * t + 2],
                                     zT[:, t * 128:(t + 1) * 128], wv_r[:],
                                     start=True, stop=True)
                wl = sbb.tile([128, 2 * NT], bf16, name=f"wl{b}", tag="wl",
                              bufs=2)
                nc.gpsimd.memset(wl[:], 1.0)
                wl3 = wl.rearrange("p (t two) -> p t two", two=2)
                pw3 = pw.rearrange("p (t two) -> p t two", two=2)
                nc.vector.tensor_scalar(wl3[:, :, 0], pw3[:, :, 0], cbo,
                                        None, ADD)
                s["wl"] = wl

            def emit_scores_tk(b, tk):
                s = st[b]
                lhs = s["zT"][:, tk * 128:(tk + 1) * 128]
                eTs = []
                for h in range(NH):
                    ps_t = ps_sc.tile([128, 1024], f32,
                                      name=f"s{b}_{tk}_{h}", tag="sc")
                    for j in range(2):
                        o = h * 1024 + j * 512
                        nc.tensor.matmul(ps_t[:, j * 512:(j + 1) * 512],
                                         lhs, s["UT"][:, o:o + 512],
                                         start=True, stop=True)
                    eT = sbe.tile([128, 1024], bf16,
                                  name=f"e{b}_{tk}_{h}", tag="eT", bufs=14)
                    nc.scalar.activation(eT[:], ps_t[:], AF.Exp, scale=SCALE)
                    eTs.append(eT)
                return eTs

            def emit_nd_tk(b, tk, eTs):
                s = st[b]
                wlt = s["wl"][:, 2 * tk:2 * tk + 2]
                for h in range(NH):
                    for j in range(2):
                        c = 2 * h + j
                        nc.tensor.matmul(
                            s["nd"][c][:], wlt,
                            eTs[h][:, j * 512:(j + 1) * 512],
                            start=(tk == 0), stop=(tk == NT - 1))

            def emit_finale(b):
                s = st[b]
                ndall = sbb.tile([2, A], f32, name=f"ndall{b}", tag="ndall", bufs=2)
                for c in range(NC_):
                    nc.vector.tensor_copy(
                        ndall[0:2, c * 512:(c + 1) * 512], s["nd"][c][:])
                # gather rows into [16,128]: partition t <- elements t*128..
                num16 = sbb.tile([16, 128], f32, name=f"num16{b}", tag="num16", bufs=2)
                den16 = sbb.tile([16, 128], f32, name=f"den16{b}", tag="den16", bufs=2)
                nc.gpsimd.dma_start(
                    num16[:, :],
                    ndall[0:1, :].rearrange("one (t p) -> one t p", p=128))
                nc.gpsimd.dma_start(
                    den16[:, :],
                    ndall[1:2, :].rearrange("one (t p) -> one t p", p=128))
                rcp = sbb.tile([16, 128], f32, name=f"rcp{b}", tag="rcp", bufs=2)
                nc.vector.reciprocal(rcp[:], den16[:])
                o16 = sbb.tile([16, 128], f32, name=f"o16{b}", tag="o16", bufs=2)
                nc.vector.tensor_tensor(o16[:], num16[:], rcp[:], MULT)
                nc.gpsimd.dma_start(
                    out_d[b:b + 1, :].rearrange("one (t p) -> one t p", p=128),
                    o16[:, :])
                st.pop(b)

            # ---- prologue: batch 0 setup (lean critical path) ----
            emit_z_dmas(0)
            emit_transpose_group(0, 0, n=4)
            emit_transpose_group(0, 1, n=4)
            emit_ut_half(0, 0)
            emit_transpose_group(0, 2, n=4)
            emit_transpose_group(0, 3, n=4)
            emit_ut_half(0, 1)
            emit_w(0)
            del st[0]["zn"]

            pend = []            # [(b, tk, eTs)] awaiting nd emission
            def flush_pend(keep):
                while len(pend) > keep:
                    pb, ptk, peTs = pend.pop(0)
                    emit_nd_tk(pb, ptk, peTs)
                    if ptk == NT - 1:
                        emit_finale(pb)
            for b in range(B_PER):
                s = st[b]
                s["nd"] = [ps_nd.tile([2, 512], f32, name=f"nd{b}_{c}",
                                      tag=f"nd{c}") for c in range(NC_)]
                nxt = b + 1 if b + 1 < B_PER else None
                for tk in range(NT):
                    eTs = emit_scores_tk(b, tk)
                    if len(pend) >= (2 if b == B_PER - 1 else 4):
                        flush_pend(0)
                    pend.append((b, tk, eTs))
                    if nxt is not None:
                        if tk == 1:
                            emit_z_dmas(nxt)
                        elif tk in (5, 7, 9, 11):
                            emit_transpose_group(nxt, (tk - 5) // 2, n=4)
                        elif tk == 12:
                            emit_ut_half(nxt, 0)
                        elif tk == 13:
                            emit_ut_half(nxt, 1)
                        elif tk == 14:
                            emit_w(nxt)
            flush_pend(0)

    nc.compile()
    return nc


def run(inputs: dict, trace: bool = False):
    _install_axon_shim()
    from concourse.bass_utils import run_bass_kernel_spmd

    z = np.asarray(inputs["z"], dtype=np.float32)
    Wq = np.asarray(inputs["Wq"], dtype=np.float64)
    bq = np.asarray(inputs["bq"], dtype=np.float64)
    Wk = np.asarray(inputs["Wk"], dtype=np.float64)
    Wv = np.asarray(inputs["Wv"], dtype=np.float64)
    bv = np.asarray(inputs["bv"], dtype=np.float64)
    Wo = np.asarray(inputs["Wo"], dtype=np.float64)
    bo = np.asarray(inputs["bo"], dtype=np.float64)

    # host-side weight algebra (tiny, exact in float64)
    m_lhs = (Wq.T @ Wk).astype(np.float32)            # [d, d']
    gw = (Wk.T @ bq).astype(np.float32).reshape(D, 1)
    wv = np.repeat((Wv.T @ Wo[0]).astype(np.float32).reshape(D, 1), 2, axis=1)
    cbo = float(bv @ Wo[0] + bo[0])

    nc = _build_program(cbo, SCORES_DTYPE == "bf16")

    in_maps = []
    for c in range(N_CORES):
        in_maps.append({
            "z": z[c * B_PER:(c + 1) * B_PER],
            "m_lhs": m_lhs,
            "gw": gw,
            "wv": wv,
        })
    res = run_bass_kernel_spmd(nc, in_maps, core_ids=list(range(N_CORES)),
                               trace=trace)
    out = np.concatenate([res.results[c]["out"] for c in range(N_CORES)],
                         axis=0)
    return out.reshape(B, A, 1).astype(np.float32), res


def kernel(**inputs) -> np.ndarray:
    out, _ = run(inputs, trace=False)
    return out


# revision 24
# speedup vs baseline: 1.0176x; 1.0176x over previous
"""Trainium2 Bass kernel for nn_CANN_75857712382071.

Single-head self-attention (B=32, A=2048, D=128) with scalar output
projection, algebraically collapsed:

    out[b,aq] = (sum_ak E * (w+c+bo)) / (sum_ak E)
    E = exp(scale * (z M z^T + 1 (x) g)),  M = Wq^T Wk
    g[ak] = z[ak] . (Wk^T bq),   w[ak] = z[ak] . (Wv^T Wo^T)

q/k/v/h are never materialized; softmax max-subtraction is skipped
(logits are O(10); softmax is shift-invariant in exact arithmetic).

Data-parallel over batch: 4 batches per core on 8 NeuronCores.
Batches are software-pipelined: batch b+1's setup (z DMA, PE
transposes to zT, UT = M zT + gw, w column) is emitted in small pieces
inside batch b's main loop so ScalarE (exp) never starves.
"""

import sys
import types

import numpy as np

N_CORES = 8
B, A, D = 32, 2048, 128
B_PER = B // N_CORES
SCALE = float(D) ** -0.5
SCORES_DTYPE = "bf16"   # "bf16" (fast) or "f32r" (more precise scores)


def _install_axon_shim():
    """Allow run_bass_kernel_spmd(trace=True) to NTFF-profile under axon."""
    try:
        import antenv  # noqa: F401
    except ImportError:
        return
    if "antenv.axon_hooks" not in sys.modules:
        mod = types.ModuleType("antenv.axon_hooks")
        _hook = [None]
        mod.set_axon_ntff_profile_hook = lambda h: _hook.__setitem__(0, h)
        mod.get_axon_ntff_profile_hook = lambda: _hook[0]
        sys.modules["antenv.axon_hooks"] = mod
    from antenv.axon_hooks import (
        get_axon_ntff_profile_hook,
        set_axon_ntff_profile_hook,
    )
    if get_axon_ntff_profile_hook() is None:
        try:
            from trn_agent_boot.trn_boot import _ntff_profile_via_ctypes
            set_axon_ntff_profile_hook(
                _ntff_profile_via_ctypes("/opt/axon/libaxon_pjrt.so"))
        except Exception:
            pass
    try:
        from concourse import bass_utils
        bass_utils.upload_artifacts = lambda tmpdir: tmpdir
    except Exception:
        pass


def _build_program(cbo: float, scores_bf16: bool):
    import concourse.bacc as bacc
    import concourse.mybir as mybir
    import concourse.tile as tile
    from concourse import masks

    f32 = mybir.dt.float32
    f32r = mybir.dt.float32r
    bf16 = mybir.dt.bfloat16
    sdt = bf16 if scores_bf16 else f32r
    AF = mybir.ActivationFunctionType
    ADD = mybir.AluOpType.add
    MULT = mybir.AluOpType.mult

    nc = bacc.Bacc("TRN2", target_bir_lowering=False, debug=False,
                   num_devices=N_CORES, num_swdge_queues=2)

    z_d = nc.dram_tensor("z", [B_PER, A, D], f32, kind="ExternalInput").ap()
    m_d = nc.dram_tensor("m_lhs", [D, D], f32, kind="ExternalInput").ap()
    gw_d = nc.dram_tensor("gw", [D, 1], f32, kind="ExternalInput").ap()
    wv_d = nc.dram_tensor("wv", [D, 2], f32, kind="ExternalInput").ap()
    out_d = nc.dram_tensor("out", [B_PER, A], f32, kind="ExternalOutput").ap()

    NT = A // 128          # 16 ak tiles / z tiles
    NH = A // 1024         # 2 aq halves (ACT op width 1024)
    NC_ = A // 512         # 4 aq chunks (nd accumulators)

    with tile.TileContext(nc) as tc:
        with (
            tc.tile_pool(name="sb", bufs=1) as sb,
            tc.tile_pool(name="ps_sc", bufs=2, space="PSUM") as ps_sc,
        ):
            ps_nd = ps_sc
            sbc = sbz = sbe = sbb = sb
            # ---- constants ----
            m_f = sbc.tile([D, D], f32)
            nc.sync.dma_start(m_f[:], m_d[:])
            gw_col = sbc.tile([D, 1], f32)
            nc.sync.dma_start(gw_col[:], gw_d[:])
            wv_f = sbc.tile([D, 2], f32)
            nc.sync.dma_start(wv_f[:], wv_d[:])
            ident = sbc.tile([D, D], f32)
            masks.make_identity(nc, ident[:])
            m_r = sbc.tile([D, D], sdt)
            nc.vector.tensor_copy(m_r[:], m_f[:])
            wv_r = sbc.tile([D, 2], sdt)
            nc.vector.tensor_copy(wv_r[:], wv_f[:])

            # ACT table warmup (overlaps first z DMAs)
            warm = sbc.tile([D, 1], f32)
            nc.scalar.activation(warm[:], gw_col[:], AF.Exp, scale=0.0)

            # PE HAM pre-warm: dummy bf16 matmuls on junk data while the
            # first z DMA is in flight, so the prologue runs at 2.4 GHz
            junk = sbc.tile([128, 512], bf16)
            nc.gpsimd.memset(junk[:, 0:8], 0.0)
            pjunk = ps_sc.tile([128, 512], f32, name="pjunk", tag="sc")
            for i in range(14):
                nc.tensor.matmul(pjunk[:], junk[:, 0:128], junk[:],
                                 start=True, stop=True)

            st = {}  # per-batch live tiles

            def emit_z_dmas(b):
                s = st.setdefault(b, {})
                s["zT"] = sbb.tile([D, A], sdt, name=f"zT{b}", tag="zT", bufs=2)
                zn = sbz.tile([128, A], f32, name=f"zn{b}", tag="zn", bufs=2)
                zsrc = z_d[b].rearrange("(t p) d -> p t d", p=128)
                zdst = zn.rearrange("p (t d) -> p t d", d=D)
                engs = (nc.sync, nc.scalar, nc.gpsimd, nc.gpsimd)
                for q in range(8):
                    engs[q % 4].dma_start(zdst[:, 2 * q:2 * q + 2],
                                          zsrc[:, 2 * q:2 * q + 2])
                s["zn"] = zn

            def emit_cast_half(b, h):
                s = st[b]
                if "znb" not in s:
                    s["znb"] = sbz.tile([128, A], bf16, name=f"znb{b}",
                                        tag="znb", bufs=2)
                nc.vector.tensor_copy(s["znb"][:, h * 1024:(h + 1) * 1024],
                                      s["zn"][:, h * 1024:(h + 1) * 1024])

            def emit_transpose_dma(b, h):
                # 8 xbar DMA transposes straight into zT (no PE, no PSUM)
                s = st[b]
                for i in range(8 * h, 8 * h + 8):
                    eng = nc.sync if i % 2 == 0 else nc.scalar
                    eng.dma_start_transpose(
                        out=s["zT"][:, i * 128:(i + 1) * 128],
                        in_=s["znb"][:, i * 128:(i + 1) * 128])

            def emit_transpose_group(b, g, n=2):
                # n transposes into one psum slot, then 1 copy
                s = st[b]
                pt = ps_sc.tile([128, 1024], f32, name=f"pt{b}_{g}", tag="sc")
                for j in range(n):
                    i = n * g + j
                    nc.tensor.transpose(pt[:, j * 128:(j + 1) * 128],
                                        s["zn"][:, i * 128:(i + 1) * 128],
                                        ident[:])
                nc.vector.tensor_copy(
                    s["zT"][:, g * n * 128:(g + 1) * n * 128],
                    pt[:, 0:n * 128])

            def emit_ut_half(b, h):
                s = st[b]
                zT = s["zT"]
                if h == 0:
                    s["UT"] = sbb.tile([D, A], sdt, name=f"UT{b}", tag="UT",
                                       bufs=2)
                UT = s["UT"]
                pu = ps_sc.tile([128, 1024], f32, name=f"pu{b}_{h}",
                                tag="sc")
                for j in range(2):
                    o = h * 1024 + j * 512
                    nc.tensor.matmul(pu[:, j * 512:(j + 1) * 512],
                                     m_r[:], zT[:, o:o + 512],
                                     start=True, stop=True)
                nc.vector.tensor_scalar(
                    UT[:, h * 1024:(h + 1) * 1024], pu[:], gw_col[:],
                    None, ADD)

            def emit_w(b):
                s = st[b]
                zT = s["zT"]
                pw = ps_sc.tile([128, 2 * NT], f32, name=f"pw{b}", tag="sc")
                for t in range(NT):
                    nc.tensor.matmul(pw[:, 2 * t:2 * t + 2],
                                     zT[:, t * 128:(t + 1) * 128], wv_r[:],
                                     start=True, stop=True)
                wl = sbb.tile([128, 2 * NT], bf16, name=f"wl{b}", tag="wl",
                              bufs=2)
                nc.gpsimd.memset(wl[:], 1.0)
                wl3 = wl.rearrange("p (t two) -> p t two", two=2)
                pw3 = pw.rearrange("p (t two) -> p t two", two=2)
                nc.vector.tensor_scalar(wl3[:, :, 0], pw3[:, :, 0], cbo,
                                        None, ADD)
                s["wl"] = wl

            def emit_scores_tk(b, tk):
                s = st[b]
                lhs = s["zT"][:, tk * 128:(tk + 1) * 128]
                eTs = []
                for h in range(NH):
                    ps_t = ps_sc.tile([128, 1024], f32,
                                      name=f"s{b}_{tk}_{h}", tag="sc")
                    for j in range(2):
                        o = h * 1024 + j * 512
                        nc.tensor.matmul(ps_t[:, j * 512:(j + 1) * 512],
                                         lhs, s["UT"][:, o:o + 512],
                                         start=True, stop=True)
                    eT = sbe.tile([128, 1024], bf16,
                                  name=f"e{b}_{tk}_{h}", tag="eT", bufs=14)
                    nc.scalar.activation(eT[:], ps_t[:], AF.Exp, scale=SCALE)
                    eTs.append(eT)
                return eTs

            def emit_nd_tk(b, tk, eTs):
                s = st[b]
                wlt = s["wl"][:, 2 * tk:2 * tk + 2]
                for h in range(NH):
                    for j in range(2):
                        c = 2 * h + j
                        nc.tensor.matmul(
                            s["nd"][c][:], wlt,
                            eTs[h][:, j * 512:(j + 1) * 512],
                            start=(tk == 0), stop=(tk == NT - 1))

            def emit_finale(b):
                s = st[b]
                ndall = sbb.tile([2, A], f32, name=f"ndall{b}", tag="ndall", bufs=2)
                for c in range(NC_):
                    nc.vector.tensor_copy(
                        ndall[0:2, c * 512:(c + 1) * 512], s["nd"][c][:])
                # gather rows into [16,128]: partition t <- elements t*128..
                num16 = sbb.tile([16, 128], f32, name=f"num16{b}", tag="num16", bufs=2)
                den16 = sbb.tile([16, 128], f32, name=f"den16{b}", tag="den16", bufs=2)
                nc.scalar.dma_start(
                    den16[:, :],
                    ndall[1:2, :].rearrange("one (t p) -> one t p", p=128))
                nc.sync.dma_start(
                    num16[:, :],
                    ndall[0:1, :].rearrange("one (t p) -> one t p", p=128))
                rcp = sbb.tile([16, 128], f32, name=f"rcp{b}", tag="rcp", bufs=2)
                nc.vector.reciprocal(rcp[:], den16[:])
                o16 = sbb.tile([16, 128], f32, name=f"o16{b}", tag="o16", bufs=2)
                nc.vector.tensor_tensor(o16[:], num16[:], rcp[:], MULT)
                nc.sync.dma_start(
                    out_d[b:b + 1, :].rearrange("one (t p) -> one t p", p=128),
                    o16[:, :])
                st.pop(b)

            # ---- prologue: batch 0 setup (lean critical path) ----
            emit_z_dmas(0)
            emit_transpose_group(0, 0, n=4)
            emit_transpose_group(0, 1, n=4)
            emit_ut_half(0, 0)
            emit_transpose_group(0, 2, n=4)
            emit_transpose_group(0, 3, n=4)
            emit_ut_half(0, 1)
            emit_w(0)
            del st[0]["zn"]

            pend = []            # [(b, tk, eTs)] awaiting nd emission
            def flush_pend(keep):
                while len(pend) > keep:
                    pb, ptk, peTs = pend.pop(0)
                    emit_nd_tk(pb, ptk, peTs)
                    if ptk == NT - 1:
                        emit_finale(pb)
            for b in range(B_PER):
                s = st[b]
                s["nd"] = [ps_nd.tile([2, 512], f32, name=f"nd{b}_{c}",
                                      tag=f"nd{c}", bufs=1)
                           for c in range(NC_)]
                nxt = b + 1 if b + 1 < B_PER else None
                for tk in range(NT):
                    eTs = emit_scores_tk(b, tk)
                    if len(pend) >= (2 if b == B_PER - 1 else 4):
                        flush_pend(0)
                    pend.append((b, tk, eTs))
                    if nxt is not None:
                        if tk == 1:
                            emit_z_dmas(nxt)
                        elif tk in (5, 7, 9, 11):
                            emit_transpose_group(nxt, (tk - 5) // 2, n=4)
                        elif tk == 12:
                            emit_ut_half(nxt, 0)
                        elif tk == 13:
                            emit_ut_half(nxt, 1)
                        elif tk == 14:
                            emit_w(nxt)
            flush_pend(0)

    nc.compile()
    return nc


def run(inputs: dict, trace: bool = False):
    _install_axon_shim()
    from concourse.bass_utils import run_bass_kernel_spmd

    z = np.asarray(inputs["z"], dtype=np.float32)
    Wq = np.asarray(inputs["Wq"], dtype=np.float64)
    bq = np.asarray(inputs["bq"], dtype=np.float64)
    Wk = np.asarray(inputs["Wk"], dtype=np.float64)
    Wv = np.asarray(inputs["Wv"], dtype=np.float64)
    bv = np.asarray(inputs["bv"], dtype=np.float64)
    Wo = np.asarray(inputs["Wo"], dtype=np.float64)
    bo = np.asarray(inputs["bo"], dtype=np.float64)

    # host-side weight algebra (tiny, exact in float64)
    m_lhs = (Wq.T @ Wk).astype(np.float32)            # [d, d']
    gw = (Wk.T @ bq).astype(np.float32).reshape(D, 1)
    wv = np.repeat((Wv.T @ Wo[0]).astype(np.float32).reshape(D, 1), 2, axis=1)
    cbo = float(bv @ Wo[0] + bo[0])

    nc = _build_program(cbo, SCORES_DTYPE == "bf16")

    in_maps = []
    for c in range(N_CORES):
        in_maps.append({
            "z": z[c * B_PER:(c + 1) * B_PER],
            "m_lhs": m_lhs,
            "gw": gw,
            "wv": wv,
        })
    res = run_bass_kernel_spmd(nc, in_maps, core_ids=list(range(N_CORES)),
                               trace=trace)
    out = np.concatenate([res.results[c]["out"] for c in range(N_CORES)],
                         axis=0)
    return out.reshape(B, A, 1).astype(np.float32), res


def kernel(**inputs) -> np.ndarray:
    out, _ = run(inputs, trace=False)
    return out
